# revision 1
# baseline (speedup 1.0000x reference)
"""Trainium2 Bass kernel for MultiLinearAttention (causal linear attention).

Reference computation (per head h, feature map phi(u) = elu(u)+1):
    q = phi(x_h @ Wq_h), k = phi(x_h @ Wk_h), v = x_h @ Wv_h
    y_t = (q_t . sum_{s<=t} k_s v_s^T) / (q_t . sum_{s<=t} k_s + eps)
    out = concat_h(y_h) @ Wp

Sharding: 16 heads / 8 cores = 2 heads per core, all 4 batches per core.
Wp is folded per-head into the v projection (W'_h = Wv_h @ Wp_h), so each
core produces a partial [B, S, 64] output summed on the host (the unshard
step for this head/output-partial sharding).

Device algorithm: chunked causal linear attention, chunk C=128:
    A^T = K_chunk Q_chunk^T (per head), masked to s<=t
    num = A_m^T V'aug + Q^T S_aug   (aug col of V' is ones -> den)
    S_aug += K_chunk^T V'aug
phi is computed as  max(u+1, min(exp(u), 1))  == elu(u)+1, with the +1
coming from presetting PSUM to 1 via a K=1 ones matmul.

Two batches are processed per instruction ("pair batching") to amortize
fixed per-instruction engine costs.
"""

import os
import sys

import numpy as np

for _p in ("/root/.axon_site/_ro/trn_rl_repo", "/opt/trn_rl_repo", "/opt/pypackages"):
    if os.path.isdir(_p) and _p not in sys.path:
        sys.path.append(_p)

import ml_dtypes

B, S, D = 4, 4096, 1024
H, HD, O = 16, 64, 64
C = 128                  # chunk length
NCORE = 8
HPC = H // NCORE         # heads per core
NCHUNK = S // C

USE_BF16 = True

_CACHE = {}


def _build_program(nchunk=NCHUNK, stage=99):
    import concourse.mybir as mybir
    from concourse import bacc
    from concourse.tile import TileContext

    fp32 = mybir.dt.float32
    cdt = mybir.dt.bfloat16 if USE_BF16 else fp32
    Alu = mybir.AluOpType
    Act = mybir.ActivationFunctionType

    nc = bacc.Bacc()
    xT_h = nc.declare_dram_parameter("xT", [B, 128, S], cdt, isOutput=False)
    wq_h = nc.declare_dram_parameter("wq", [128, 128], cdt, isOutput=False)
    wk_h = nc.declare_dram_parameter("wk", [128, 128], cdt, isOutput=False)
    wv_h = nc.declare_dram_parameter("wv", [128, 128], cdt, isOutput=False)
    mask_h = nc.declare_dram_parameter("mask2", [128, 512], cdt, isOutput=False)
    ident_h = nc.declare_dram_parameter("ident", [128, 128], cdt, isOutput=False)
    ones_h = nc.declare_dram_parameter("ones", [1, 512], cdt, isOutput=False)
    zer_h = nc.declare_dram_parameter("zer", [1, 512], cdt, isOutput=False)
    out_h = nc.declare_dram_parameter("out", [B, S, O], fp32, isOutput=True)

    with TileContext(nc) as tc:
        with (
            tc.tile_pool(name="consts", bufs=1) as consts,
            tc.tile_pool(name="work", bufs=4) as work,
            tc.tile_pool(name="st_sb", bufs=3) as st_sb,
            tc.tile_pool(name="pu", bufs=1, space="PSUM") as pu,
            tc.tile_pool(name="pa", bufs=1, space="PSUM") as pa,
            tc.tile_pool(name="pvk", bufs=1, space="PSUM") as pvk,
            tc.tile_pool(name="pkn", bufs=1, space="PSUM") as pkn,
            tc.tile_pool(name="pnum", bufs=1, space="PSUM") as pnum,
            tc.tile_pool(name="pst", bufs=1, space="PSUM") as pst,
        ):
            # ---- constants into SBUF ----
            neg1 = consts.tile([128, 1], fp32)
            nc.gpsimd.memset(neg1, -1.0)
            wq = consts.tile([128, 128], cdt)
            wk = consts.tile([128, 128], cdt)
            wv = consts.tile([128, 128], cdt)
            mask2 = consts.tile([128, 512], cdt)
            ident = consts.tile([128, 128], cdt)
            ones = consts.tile([1, 512], cdt)
            zer = consts.tile([1, 512], cdt)
            nc.sync.dma_start(wq, wq_h[:, :])
            nc.sync.dma_start(wk, wk_h[:, :])
            nc.sync.dma_start(wv, wv_h[:, :])
            nc.sync.dma_start(mask2, mask_h[:, :])
            nc.sync.dma_start(ident, ident_h[:, :])
            nc.sync.dma_start(ones, ones_h[:, :])
            nc.sync.dma_start(zer, zer_h[:, :])

            xsb = []
            for b in range(B):
                xb = consts.tile([128, S], cdt, name=f"xsb{b}")
                nc.sync.dma_start(xb, xT_h[b])
                xsb.append(xb)

            # persistent per-pair state PSUM: [S'(b_even) | S'(b_odd)], each
            # [128, 130] with head0 block [0:64, 0:65], head1 [64:128, 65:130]
            st_ps = [
                pst.tile([128, 260], fp32, name="stA"),
                pst.tile([128, 260], fp32, name="stB"),
            ]
            # One start=True zero-write owns each state bank; all later state
            # matmuls accumulate with start=False. (start=True marks the whole
            # 2KB PSUM zero-region pending, so it must appear exactly once.)
            for stp in st_ps:
                nc.tensor.matmul(stp, ones[:, 0:128], zer[:, 0:260],
                                 start=True, stop=False, skip_group_check=True)

            s01_prev = [None, None]

            for i in range(nchunk):
                sl = slice(i * C, (i + 1) * C)
                for pr in range(2):
                    b0, b1 = 2 * pr, 2 * pr + 1
                    stp = st_ps[pr]

                    # ---------------- PE: projections ----------------
                    # u layout: [q(b0) | k(b0) | q(b1) | k(b1)] each [128,128]
                    u = pu.tile([128, 512], fp32, name="u")
                    # preset PSUM to 1.0 so u holds w = proj + 1
                    nc.tensor.matmul(u, ones[:, 0:128], ones, start=True,
                                     stop=False, skip_group_check=True)
                    for j, xb in enumerate((xsb[b0], xsb[b1])):
                        nc.tensor.matmul(u[:, 256 * j:256 * j + 128], wq,
                                         xb[:, sl], start=False, stop=True,
                                         skip_group_check=True)
                    for j, xb in enumerate((xsb[b0], xsb[b1])):
                        nc.tensor.matmul(u[:, 256 * j + 128:256 * j + 256], wk,
                                         xb[:, sl], start=False, stop=True,
                                         skip_group_check=True)
                    # vk psum: [v'(b0) | v'(b1)]
                    vk = pvk.tile([128, 256], fp32, name="vk")
                    for j, xb in enumerate((xsb[b0], xsb[b1])):
                        nc.tensor.matmul(vk[:, 128 * j:128 * (j + 1)],
                                         xb[:, sl], wv, start=True, stop=True)

                    # ---------------- phi ----------------
                    # e = exp(w - 1) = exp(u);  phi = max(w, min(e, 1))
                    e2 = work.tile([128, 512], cdt, name="e2")
                    nc.scalar.activation(e2, u, Act.Exp, bias=neg1)
                    phi2 = work.tile([128, 512], cdt, name="phi2")
                    nc.vector.scalar_tensor_tensor(
                        phi2, e2, 1.0, u, Alu.min, Alu.max)

                    if stage < 1:
                        continue
                    # ---------------- A^T = K Q^T per (b, h) ----------------
                    # One PSUM bank per head: all matmuls writing a given bank
                    # must read operands from the same base partition (HW).
                    ah = [pa.tile([128, 256], fp32, name="a_h0"),
                          pa.tile([128, 256], fp32, name="a_h1")]
                    for j in range(2):
                        qq = phi2[:, 256 * j:256 * j + 128]
                        kk = phi2[:, 256 * j + 128:256 * j + 256]
                        for h in range(2):
                            es = slice(64 * h, 64 * (h + 1))
                            nc.tensor.matmul(
                                ah[h][:, 128 * j:128 * (j + 1)],
                                kk[es, :], qq[es, :], start=True, stop=True)

                    if stage < 2:
                        continue
                    # knat via PE transpose of phi(k) into bf16 PSUM
                    knp = pkn.tile([128, 256], cdt, name="knp")
                    for j in range(2):
                        nc.tensor.transpose(
                            knp[:, 128 * j:128 * (j + 1)],
                            phi2[:, 256 * j + 128:256 * j + 256], ident)

                    # masked A -> SBUF; layout [b0h0 | b1h0 | b0h1 | b1h1]
                    am2 = work.tile([128, 512], cdt, name="am2")
                    nc.vector.tensor_tensor(am2[:, 0:256], ah[0], mask2[:, 0:256],
                                            Alu.mult)
                    nc.vector.tensor_tensor(am2[:, 256:512], ah[1],
                                            mask2[:, 256:512], Alu.mult)

                    # V'aug: [v0 | 1 | v1 | 1] per b  (130 cols per b)
                    vaug = work.tile([128, 260], cdt, name="vaug")
                    vsrc = vk.rearrange("p (g c) -> p g c", c=64)
                    vdst = vaug.rearrange("p (g c) -> p g c", c=65)[:, :, 0:64]
                    nc.scalar.copy(vdst, vsrc)
                    vones = vaug.rearrange("p (g c) -> p g c", c=65)[:, :, 64:65]
                    nc.gpsimd.memset(vones, 1.0)

                    knat2 = work.tile([128, 256], cdt, name="knat2")
                    nc.vector.tensor_copy(knat2, knp)

                    if stage < 3:
                        continue
                    # ---------------- num = A_m^T Vaug + Q^T S ----------------
                    num = pnum.tile([128, 260], fp32, name="num")
                    nc.tensor.matmul(num, ones[:, 0:128], zer[:, 0:260],
                                     start=True, stop=False,
                                     skip_group_check=True)
                    # For b1, head blocks are stored swapped (h1 first) so the
                    # h1 state block (output partition offset 64) lands at a
                    # column where its AP stays within one PSUM bank. Heads
                    # are summed at the end, so block identity is positional.
                    for j in range(2):
                        for h in range(2):
                            hp = h ^ j  # head's positional slot
                            reg = slice(130 * j + 65 * hp, 130 * j + 65 * (hp + 1))
                            va = vaug[:, 130 * j + 65 * h:130 * j + 65 * (h + 1)]
                            nc.tensor.matmul(
                                num[:, reg],
                                am2[:, 256 * h + 128 * j:256 * h + 128 * (j + 1)],
                                va, start=False, stop=False,
                                skip_group_check=True)
                        if i > 0:
                            # both heads at once: K=128 with block-diag state
                            sp = s01_prev[pr]
                            nc.tensor.matmul(
                                num[:, 130 * j:130 * (j + 1)],
                                phi2[:, 256 * j:256 * j + 128],
                                sp[:, 130 * j:130 * (j + 1)],
                                start=False, stop=True,
                                skip_group_check=True)

                    if stage < 4:
                        continue
                    # ---------------- state update (diag blocks only) --------
                    # Per-head matmuls with base-0 operands; h1 writes at
                    # output partition offset 64. Off-diag blocks stay zero so
                    # the s01 copy is directly block-diagonal.
                    for j in range(2):
                        for h in range(2):
                            hp = h ^ j
                            nc.tensor.matmul(
                                stp[64 * h:64 * (h + 1),
                                    130 * j + 65 * hp:130 * j + 65 * (hp + 1)],
                                knat2[:, 128 * j + 64 * h:128 * j + 64 * (h + 1)],
                                vaug[:, 130 * j + 65 * h:130 * j + 65 * (h + 1)],
                                start=False, stop=False,
                                skip_group_check=True)

                    if i < NCHUNK - 1:
                        s01 = st_sb.tile([128, 260], cdt, name="s01")
                        nc.scalar.copy(s01, stp)
                        s01_prev[pr] = s01

                    if stage < 5:
                        continue
                    # ---------------- y = num/den, sum heads ----------------
                    rec = work.tile([128, 4], fp32, name="rec")
                    dens = num.rearrange("p (g c) -> p g c", c=65)[:, :, 64:65]
                    nc.vector.reciprocal(rec, dens)
                    for j, b in enumerate((b0, b1)):
                        y1 = work.tile([128, 64], fp32, name=f"y1_{j}")
                        nc.vector.tensor_scalar_mul(
                            y1, num[:, 130 * j + 65:130 * j + 129],
                            rec[:, 2 * j + 1:2 * j + 2])
                        yo = work.tile([128, 64], fp32, name=f"yo_{j}")
                        nc.vector.scalar_tensor_tensor(
                            yo, num[:, 130 * j:130 * j + 64],
                            rec[:, 2 * j:2 * j + 1], y1, Alu.mult, Alu.add)
                        nc.sync.dma_start(out_h[b, sl, :], yo)

    nc.finalize()
    return nc


def _host_prep(x, Wq, Wk, Wv, Wp):
    """Shard inputs per core; returns in_maps list."""
    x = np.asarray(x, dtype=np.float32)
    Wq = np.asarray(Wq, dtype=np.float32)
    Wk = np.asarray(Wk, dtype=np.float32)
    Wv = np.asarray(Wv, dtype=np.float32)
    Wp = np.asarray(Wp, dtype=np.float32)
    ndt = ml_dtypes.bfloat16 if USE_BF16 else np.float32

    mask = np.triu(np.ones((C, C), np.float32))
    mask2 = np.tile(mask, (1, 4)).astype(ndt)          # [128, 512]
    ident = np.eye(128, dtype=np.float32).astype(ndt)
    ones = np.ones((1, 512), np.float32).astype(ndt)

    in_maps = []
    for c in range(NCORE):
        h0 = HPC * c
        xs = x[:, :, 64 * h0:64 * (h0 + HPC)]          # [B, S, 128]
        xT = np.ascontiguousarray(xs.transpose(0, 2, 1)).astype(ndt)
        wq_bd = np.zeros((128, 128), np.float32)
        wk_bd = np.zeros((128, 128), np.float32)
        wv_bd = np.zeros((128, 128), np.float32)
        for j in range(HPC):
            h = h0 + j
            sl = slice(64 * j, 64 * (j + 1))
            wq_bd[sl, sl] = Wq[h]
            wk_bd[sl, sl] = Wk[h]
            wv_bd[sl, sl] = Wv[h] @ Wp[64 * h:64 * (h + 1), :]
        in_maps.append({
            "xT": xT,
            "wq": wq_bd.astype(ndt),
            "wk": wk_bd.astype(ndt),
            "wv": wv_bd.astype(ndt),
            "mask2": mask2,
            "ident": ident,
            "ones": ones,
            "zer": np.zeros((1, 512), np.float32).astype(ndt),
        })
    return in_maps


def get_program():
    if "nc" not in _CACHE:
        _CACHE["nc"] = _build_program()
    return _CACHE["nc"]


def run_spmd(in_maps, **kwargs):
    from concourse.bass_utils import run_bass_kernel_spmd
    nc = get_program()
    return run_bass_kernel_spmd(nc, in_maps, list(range(NCORE)), **kwargs)


def kernel(x, Wq, Wk, Wv, Wp):
    in_maps = _host_prep(x, Wq, Wk, Wv, Wp)
    res = run_spmd(in_maps)
    out = np.zeros((B, S, O), np.float32)
    for c in range(NCORE):
        out += res.results[c]["out"]
    return out



# revision 2
# speedup vs baseline: 1.3204x; 1.3204x over previous
"""Trainium2 Bass kernel for MultiLinearAttention (causal linear attention).

Reference computation (per head h, feature map phi(u) = elu(u)+1):
    q = phi(x_h @ Wq_h), k = phi(x_h @ Wk_h), v = x_h @ Wv_h
    y_t = (q_t . sum_{s<=t} k_s v_s^T) / (q_t . sum_{s<=t} k_s + eps)
    out = concat_h(y_h) @ Wp

Sharding: 16 heads / 8 cores = 2 heads per core, all 4 batches per core.
Wp is folded per-head into the v projection (W'_h = Wv_h @ Wp_h), so each
core produces per-head numerators num_h and denominators den_h; the host
computes sum_h num_h/den_h and sums partials over cores (the unshard step).

Device algorithm per chunk of C=128 (all 4 batches in one pass):
    u_q = Wq^T x + 1, u_k = Wk^T x + 1      (+1 from PSUM preset matmul)
    phi = max(u, min(exp(u-1), 1))          == elu(proj)+1
    A^T = K Q^T per (b,h), masked to s<=t   (mask on DVE)
    num = A_m^T [v|1] + Q^T [S|z]           (aug col gives den)
    S  += K_chunk^T V (PSUM-persistent), z += sum_s k_s (accum_out)
num/den are DMA'd out in bf16; the final division happens on the host.
"""

import os
import sys

import numpy as np

for _p in ("/root/.axon_site/_ro/trn_rl_repo", "/opt/trn_rl_repo", "/opt/pypackages"):
    if os.path.isdir(_p) and _p not in sys.path:
        sys.path.append(_p)

import ml_dtypes

B, S, D = 4, 4096, 1024
H, HD, O = 16, 64, 64
C = 128                  # chunk length
NCORE = 8
HPC = H // NCORE         # heads per core
NCHUNK = S // C
NSLAB = 8                # x is DMA'd in NSLAB slabs of NCHUNK//NSLAB chunks
CPS = NCHUNK // NSLAB    # chunks per slab

_CACHE = {}


def _build_program(nchunk=NCHUNK):
    import concourse.mybir as mybir
    from concourse import bacc
    from concourse.tile import TileContext

    fp32 = mybir.dt.float32
    bf16 = mybir.dt.bfloat16
    Alu = mybir.AluOpType
    Act = mybir.ActivationFunctionType

    nc = bacc.Bacc()
    xq_h = nc.declare_dram_parameter("xq", [NSLAB, 128, CPS * 512], bf16,
                                     isOutput=False)
    wq_h = nc.declare_dram_parameter("wq", [128, 128], bf16, isOutput=False)
    wk_h = nc.declare_dram_parameter("wk", [128, 128], bf16, isOutput=False)
    wv_h = nc.declare_dram_parameter("wv", [128, 128], bf16, isOutput=False)
    mask_h = nc.declare_dram_parameter("mask2", [128, 512], bf16, isOutput=False)
    ident_h = nc.declare_dram_parameter("ident", [128, 128], bf16, isOutput=False)
    ones_h = nc.declare_dram_parameter("ones", [1, 512], bf16, isOutput=False)
    zer_h = nc.declare_dram_parameter("zer", [1, 512], bf16, isOutput=False)
    # out[pr, i] = [t, 260]: cols 130*j + [h0 num 64 | h0 den | h1 num 64
    # | h1 den] for batch b = 2*pr + j
    out_h = nc.declare_dram_parameter("out", [2, nchunk, 128, 260], bf16,
                                      isOutput=True)

    with TileContext(nc) as tc:
        with (
            tc.tile_pool(name="consts", bufs=1) as consts,
            tc.tile_pool(name="work", bufs=2) as work,
            tc.tile_pool(name="nouts", bufs=2) as nouts,
            tc.tile_pool(name="pq", bufs=1, space="PSUM") as pq,
            tc.tile_pool(name="pk", bufs=1, space="PSUM") as pk,
            tc.tile_pool(name="pab", bufs=1, space="PSUM") as pab,
            tc.tile_pool(name="pa1", bufs=1, space="PSUM") as pa1,
            tc.tile_pool(name="pknp", bufs=1, space="PSUM") as pknp,
            tc.tile_pool(name="pn0", bufs=1, space="PSUM") as pn0,
            tc.tile_pool(name="pn1", bufs=1, space="PSUM") as pn1,
            tc.tile_pool(name="pst", bufs=1, space="PSUM") as pst,
        ):
            # ---- constants into SBUF ----
            neg1 = consts.tile([128, 1], fp32)
            nc.gpsimd.memset(neg1, -1.0)
            wq = consts.tile([128, 128], bf16)
            wk = consts.tile([128, 128], bf16)
            wv = consts.tile([128, 128], bf16)
            mask2 = consts.tile([128, 512], bf16)
            ident = consts.tile([128, 128], bf16)
            ones = consts.tile([1, 512], bf16)
            zer = consts.tile([1, 512], bf16)
            nc.sync.dma_start(wq, wq_h[:, :])
            nc.sync.dma_start(wk, wk_h[:, :])
            nc.sync.dma_start(wv, wv_h[:, :])
            nc.sync.dma_start(mask2, mask_h[:, :])
            nc.sync.dma_start(ident, ident_h[:, :])
            nc.sync.dma_start(ones, ones_h[:, :])
            nc.sync.dma_start(zer, zer_h[:, :])

            xq = consts.tile([128, NSLAB * CPS * 512], bf16, name="xq")
            for sb in range(NSLAB):
                nc.sync.dma_start(
                    xq[:, sb * CPS * 512:(sb + 1) * CPS * 512], xq_h[sb])

            # persistent SBUF state
            vaug = consts.tile([128, 520], bf16, name="vaug")
            nc.gpsimd.memset(vaug, 1.0)   # aug cols stay 1.0 forever
            s01 = [consts.tile([128, 520], bf16, name=f"s01{j}")
                   for j in range(2)]
            for t in s01:
                nc.gpsimd.memset(t, 0.0)  # z off-blocks must stay 0
            zsb = [consts.tile([128, 4], fp32, name=f"z{j}") for j in range(2)]
            nc.gpsimd.memset(zsb[0], 0.0)
            nc.gpsimd.memset(zsb[1], 0.0)

            # persistent state PSUM bank: per b block cols 128b..128b+128:
            # h0 at [0:64, 128b:128b+64], h1 at [64:128, 128b+64:128b+128]
            st = pst.tile([128, 512], fp32, name="st")
            nc.tensor.matmul(st, ones[:, 0:128], zer[:, 0:512],
                             start=True, stop=False, skip_group_check=True)

            for i in range(nchunk):
                last = i == nchunk - 1
                xc = (i // CPS) * (CPS * 512) + (i % CPS) * 512
                xslab = xq[:, xc:xc + 512]          # [feat, (b,t)]

                # ---------------- PE: projections ----------------
                u_k = pk.tile([128, 512], fp32, name="u_k")
                nc.tensor.matmul(u_k, ones[:, 0:128], ones[:, 0:512],
                                 start=True, stop=False, skip_group_check=True)
                nc.tensor.matmul(u_k, wk, xslab, start=False, stop=True,
                                 skip_group_check=True)
                u_q = pq.tile([128, 512], fp32, name="u_q")
                nc.tensor.matmul(u_q, ones[:, 0:128], ones[:, 0:512],
                                 start=True, stop=False, skip_group_check=True)
                nc.tensor.matmul(u_q, wq, xslab, start=False, stop=True,
                                 skip_group_check=True)
                # v time-major into the shared vk/ah0 bank
                vkab = pab.tile([128, 512], fp32, name="vkab")
                for b in range(B):
                    nc.tensor.matmul(vkab[:, 128 * b:128 * (b + 1)],
                                     xslab[:, 128 * b:128 * (b + 1)], wv,
                                     start=(b == 0), stop=(b == 3),
                                     skip_group_check=True)

                # ---------------- phi ----------------
                e_k = work.tile([128, 512], bf16, name="e_k")
                nc.scalar.activation(e_k, u_k, Act.Exp, bias=neg1)
                e_q = work.tile([128, 512], bf16, name="e_q")
                nc.scalar.activation(e_q, u_q, Act.Exp, bias=neg1)
                phi_k = work.tile([128, 512], bf16, name="phi_k")
                zch = work.tile([128, 4], fp32, name="zch")
                for b in range(B):
                    bs = slice(128 * b, 128 * (b + 1))
                    nc.vector.scalar_tensor_tensor(
                        phi_k[:, bs], e_k[:, bs], 1.0, u_k[:, bs],
                        Alu.min, Alu.max,
                        accum_out=None if last else zch[:, b:b + 1])
                phi_q = work.tile([128, 512], bf16, name="phi_q")
                nc.vector.scalar_tensor_tensor(
                    phi_q, e_q, 1.0, u_q, Alu.min, Alu.max)

                # vaug <- v (aug ones cols untouched)
                vsrc = vkab.rearrange("p (g c) -> p g c", c=64)
                vdst = vaug.rearrange("p (g c) -> p g c", c=65)[:, :, 0:64]
                nc.scalar.copy(vdst, vsrc)

                # ---------------- knat via PE transpose ----------------
                if not last:
                    knp = pknp.tile([128, 512], bf16, name="knp")
                    for b in range(B):
                        bs = slice(128 * b, 128 * (b + 1))
                        nc.tensor.transpose(knp[:, bs], phi_k[:, bs], ident)

                # ---------------- A^T = K Q^T per (b, h) ----------------
                ah = [vkab, pa1.tile([128, 512], fp32, name="ah1")]
                for h in range(2):
                    es = slice(64 * h, 64 * (h + 1))
                    for b in range(B):
                        bs = slice(128 * b, 128 * (b + 1))
                        nc.tensor.matmul(ah[h][:, bs], phi_k[es, bs],
                                         phi_q[es, bs], start=(b == 0),
                                         stop=(b == 3), skip_group_check=True)

                if not last:
                    knat = work.tile([128, 512], bf16, name="knat")
                    nc.vector.tensor_copy(knat, knp)

                am2 = [work.tile([128, 512], bf16, name=f"am2_{h}")
                       for h in range(2)]
                for h in range(2):
                    nc.vector.tensor_tensor(am2[h], ah[h], mask2, Alu.mult)

                # ---------------- num banks ----------------
                num = [pn0.tile([128, 260], fp32, name="num0"),
                       pn1.tile([128, 260], fp32, name="num1")]
                if i > 0:
                    # state-read: num[t, o'] += sum_e q[e,t] [S|z][e, o']
                    sp = s01[(i + 1) % 2]
                    for b in range(B):
                        nc.tensor.matmul(
                            num[b // 2][:, 130 * (b % 2):130 * (b % 2) + 130],
                            phi_q[:, 128 * b:128 * (b + 1)],
                            sp[:, 130 * b:130 * (b + 1)],
                            start=(b % 2 == 0), stop=False,
                            skip_group_check=True)
                # in-chunk: num += A_m^T [v | 1]
                for b in range(B):
                    for h in range(2):
                        co = 130 * (b % 2) + 65 * h
                        nc.tensor.matmul(
                            num[b // 2][:, co:co + 65],
                            am2[h][:, 128 * b:128 * (b + 1)],
                            vaug[:, 130 * b + 65 * h:130 * b + 65 * h + 65],
                            start=(i == 0 and h == 0 and b % 2 == 0),
                            stop=(h == 1 and b % 2 == 1),
                            skip_group_check=True)

                # ---------------- state update ----------------
                if not last:
                    for b in range(B):
                        for h in range(2):
                            nc.tensor.matmul(
                                st[64 * h:64 * (h + 1),
                                   128 * b + 64 * h:128 * b + 64 * h + 64],
                                knat[:, 128 * b + 64 * h:128 * b + 64 * h + 64],
                                vaug[:, 130 * b + 65 * h:130 * b + 65 * h + 64],
                                start=False, stop=False, skip_group_check=True)
                    # evacuate state + z for next chunk's state-read
                    sc = s01[i % 2]
                    sdst = sc.rearrange("p (g c) -> p g c", c=65)[:, :, 0:64]
                    ssrc = st.rearrange("p (g c) -> p g c", c=64)
                    nc.scalar.copy(sdst, ssrc)
                    zn = zsb[i % 2]
                    nc.vector.tensor_tensor(zn, zsb[(i + 1) % 2], zch, Alu.add)
                    sc3 = sc.rearrange("p (g c) -> p g c", c=130)
                    nc.gpsimd.tensor_copy(sc3[0:64, :, 64], zn[0:64, :])
                    nc.gpsimd.tensor_copy(sc3[64:128, :, 129], zn[64:128, :])

                # ---------------- export num/den ----------------
                for pr in range(2):
                    nout = nouts.tile([128, 260], bf16, name=f"no{pr}")
                    nc.scalar.copy(nout, num[pr])
                    nc.sync.dma_start(out_h[pr, i], nout)

    nc.finalize()
    return nc


def _host_prep(x, Wq, Wk, Wv, Wp):
    """Shard inputs per core; returns in_maps list."""
    x = np.asarray(x, dtype=np.float32)
    Wq = np.asarray(Wq, dtype=np.float32)
    Wk = np.asarray(Wk, dtype=np.float32)
    Wv = np.asarray(Wv, dtype=np.float32)
    Wp = np.asarray(Wp, dtype=np.float32)
    bf = ml_dtypes.bfloat16

    mask2 = np.tile(np.triu(np.ones((C, C), np.float32)), (1, 4)).astype(bf)
    ident = np.eye(128, dtype=np.float32).astype(bf)
    ones = np.ones((1, 512), np.float32).astype(bf)
    zer = np.zeros((1, 512), np.float32).astype(bf)

    in_maps = []
    for c in range(NCORE):
        h0 = HPC * c
        xs = x[:, :, 64 * h0:64 * (h0 + HPC)]          # [B, S, 128]
        # xq[slab, feat, (lc, b, t)]
        xqa = xs.reshape(B, NSLAB, CPS, C, 128).transpose(1, 4, 2, 0, 3)
        xqa = np.ascontiguousarray(xqa).reshape(NSLAB, 128, CPS * 512)
        wq_bd = np.zeros((128, 128), np.float32)
        wk_bd = np.zeros((128, 128), np.float32)
        wv_bd = np.zeros((128, 128), np.float32)
        for j in range(HPC):
            h = h0 + j
            sl = slice(64 * j, 64 * (j + 1))
            wq_bd[sl, sl] = Wq[h]
            wk_bd[sl, sl] = Wk[h]
            wv_bd[sl, sl] = Wv[h] @ Wp[64 * h:64 * (h + 1), :]
        in_maps.append({
            "xq": xqa.astype(bf),
            "wq": wq_bd.astype(bf),
            "wk": wk_bd.astype(bf),
            "wv": wv_bd.astype(bf),
            "mask2": mask2,
            "ident": ident,
            "ones": ones,
            "zer": zer,
        })
    return in_maps


def get_program():
    if "nc" not in _CACHE:
        _CACHE["nc"] = _build_program()
    return _CACHE["nc"]


def run_spmd(in_maps, **kwargs):
    from concourse.bass_utils import run_bass_kernel_spmd
    nc = get_program()
    return run_bass_kernel_spmd(nc, in_maps, list(range(NCORE)), **kwargs)


def kernel(x, Wq, Wk, Wv, Wp):
    in_maps = _host_prep(x, Wq, Wk, Wv, Wp)
    res = run_spmd(in_maps)
    out = np.zeros((B, S, O), np.float32)
    for c in range(NCORE):
        raw = np.asarray(res.results[c]["out"], dtype=np.float32)
        # raw[pr, i, t, 130j + (num_h0 64 | den_h0 | num_h1 64 | den_h1)]
        for pr in range(2):
            for j in range(2):
                b = 2 * pr + j
                nb = raw[pr, :, :, 130 * j:130 * (j + 1)].reshape(S, 130)
                out[b] += (nb[:, 0:64] / nb[:, 64:65]
                           + nb[:, 65:129] / nb[:, 129:130])
    return out


# revision 5
# speedup vs baseline: 1.6413x; 1.2431x over previous
"""Trainium2 Bass kernel for MultiLinearAttention (causal linear attention).

Reference computation (per head h, feature map phi(u) = elu(u)+1):
    q = phi(x_h @ Wq_h), k = phi(x_h @ Wk_h), v = x_h @ Wv_h
    y_t = (q_t . sum_{s<=t} k_s v_s^T) / (q_t . sum_{s<=t} k_s + eps)
    out = concat_h(y_h) @ Wp

Sharding: 16 heads / 8 cores = 2 heads per core, all 4 batches per core.
Wp is folded per-head into the v projection (W'_h = Wv_h @ Wp_h), so each
core produces per-head numerators num_h and denominators den_h; the host
computes sum_h num_h/den_h and sums partials over cores (the unshard step).

Device algorithm per chunk of C=128 (all 4 batches in one pass):
    u = W^T x  (PE);  e = exp(u), w = u+1  (Scalar/Vector)
    phi = max(min(e,1), w) == elu(proj)+1  (DVE, SBUF bf16 2x mode)
    A^T = K Q^T per (b,h) (PE, row-tiled by head), masked to s<=t (DVE)
    num = A_m^T [v|1] + Q_b^T [S|z]_b      (aug col gives den)
    S_b += K_b^T V_b: one matmul per batch over both heads; the cross-head
      blocks are garbage, zeroed by a block-mask multiply (GpSimd) during
      the PSUM->SBUF state evacuation. z is tracked via accum_out on the
      phi(k) ops and inserted into the evacuated state (GpSimd).
num/den are DMA'd out in bf16; the final division happens on the host.

HW constraint honored throughout: every matmul writing a given PSUM bank
reads its operands from the same base partition (mixing base 0 and base 64
writers in one bank faults on hardware; CoreSim does not model it).
"""

import os
import sys

import numpy as np

for _p in ("/root/.axon_site/_ro/trn_rl_repo", "/opt/trn_rl_repo", "/opt/pypackages"):
    if os.path.isdir(_p) and _p not in sys.path:
        sys.path.append(_p)

import ml_dtypes

B, S, D = 4, 4096, 1024
H, HD, O = 16, 64, 64
C = 128                  # chunk length
NCORE = 8
HPC = H // NCORE         # heads per core
NCHUNK = S // C
NSLAB = 8                # x is DMA'd in NSLAB slabs of NCHUNK//NSLAB chunks
CPS = NCHUNK // NSLAB    # chunks per slab

_CACHE = {}


def _build_program(nchunk=NCHUNK):
    import concourse.mybir as mybir
    from concourse import bacc
    from concourse.tile import TileContext

    fp32 = mybir.dt.float32
    bf16 = mybir.dt.bfloat16
    Alu = mybir.AluOpType
    Act = mybir.ActivationFunctionType

    nc = bacc.Bacc()
    xq_h = nc.declare_dram_parameter("xq", [NSLAB, 128, CPS * 512], bf16,
                                     isOutput=False)
    wq_h = nc.declare_dram_parameter("wq", [128, 128], bf16, isOutput=False)
    wk_h = nc.declare_dram_parameter("wk", [128, 128], bf16, isOutput=False)
    wv_h = nc.declare_dram_parameter("wv", [128, 128], bf16, isOutput=False)
    mask_h = nc.declare_dram_parameter("mask2", [128, 512], bf16, isOutput=False)
    bmask_h = nc.declare_dram_parameter("bmask", [128, 512], bf16, isOutput=False)
    ident_h = nc.declare_dram_parameter("ident", [128, 128], bf16, isOutput=False)
    ones_h = nc.declare_dram_parameter("ones", [1, 512], bf16, isOutput=False)
    zer_h = nc.declare_dram_parameter("zer", [1, 512], bf16, isOutput=False)
    # out[pr, i] = [t, 260]: cols 130*j + [h0 num 64 | h0 den | h1 num 64
    # | h1 den] for batch b = 2*pr + j
    out_h = nc.declare_dram_parameter("out", [2, nchunk, 128, 260], bf16,
                                      isOutput=True)

    with TileContext(nc) as tc:
        with (
            tc.tile_pool(name="consts", bufs=1) as consts,
            tc.tile_pool(name="work", bufs=2) as work,
            tc.tile_pool(name="nouts", bufs=2) as nouts,
            tc.tile_pool(name="pk", bufs=1, space="PSUM") as pk,
            tc.tile_pool(name="pq", bufs=1, space="PSUM") as pq,
            tc.tile_pool(name="pvk", bufs=1, space="PSUM") as pvk,
            tc.tile_pool(name="pa1", bufs=1, space="PSUM") as pa1,
            tc.tile_pool(name="pknp", bufs=1, space="PSUM") as pknp,
            tc.tile_pool(name="pn0", bufs=1, space="PSUM") as pn0,
            tc.tile_pool(name="pn1", bufs=1, space="PSUM") as pn1,
            tc.tile_pool(name="pst", bufs=1, space="PSUM") as pst,
        ):
            # ---- constants into SBUF ----
            wq = consts.tile([128, 128], bf16)
            wk = consts.tile([128, 128], bf16)
            wv = consts.tile([128, 128], bf16)
            mask2 = consts.tile([128, 512], bf16)
            bmask = consts.tile([128, 512], bf16)
            ident = consts.tile([128, 128], bf16)
            ones = consts.tile([1, 512], bf16)
            zer = consts.tile([1, 512], bf16)
            nc.sync.dma_start(wq, wq_h[:, :])
            nc.sync.dma_start(wk, wk_h[:, :])
            nc.sync.dma_start(wv, wv_h[:, :])
            nc.sync.dma_start(mask2, mask_h[:, :])
            nc.sync.dma_start(bmask, bmask_h[:, :])
            nc.sync.dma_start(ident, ident_h[:, :])
            nc.sync.dma_start(ones, ones_h[:, :])
            nc.sync.dma_start(zer, zer_h[:, :])

            xsl = []
            for sb in range(NSLAB):
                t = consts.tile([128, CPS * 512], bf16, name=f"xq{sb}")
                nc.sync.dma_start(t, xq_h[sb])
                xsl.append(t)

            # persistent SBUF
            vaug = consts.tile([128, 520], bf16, name="vaug")
            nc.gpsimd.memset(vaug, 1.0)   # aug cols stay 1.0 forever
            # s01[j]: evacuated state, per b cols 130b + [S_h0|z_h0|S_h1|z_h1]
            # (block-diagonal per head; off-blocks stay zero)
            s01 = [consts.tile([128, 520], bf16, name=f"s01{j}")
                   for j in range(2)]
            for t in s01:
                nc.gpsimd.memset(t, 0.0)
            stmp = consts.tile([128, 512], fp32, name="stmp")
            zsb = [consts.tile([128, 4], fp32, name=f"z{j}") for j in range(2)]
            nc.gpsimd.memset(zsb[0], 0.0)
            nc.gpsimd.memset(zsb[1], 0.0)

            # persistent state PSUM bank [e, (b: h0 64 | h1 64)]; cross-head
            # blocks hold garbage (masked out during evacuation)
            st = pst.tile([128, 512], fp32, name="st")
            nc.tensor.matmul(st, ones[:, 0:128], zer[:, 0:512],
                             start=True, stop=False, skip_group_check=True)

            for i in range(nchunk):
                last = i == nchunk - 1
                xslab = xsl[i // CPS][:, (i % CPS) * 512:(i % CPS + 1) * 512]

                # ---------------- PE: projections ----------------
                u_k = pk.tile([128, 512], fp32, name="u_k")
                nc.tensor.matmul(u_k, wk, xslab, start=True, stop=True,
                                 skip_group_check=True)
                u_q = pq.tile([128, 512], fp32, name="u_q")
                nc.tensor.matmul(u_q, wq, xslab, start=True, stop=True,
                                 skip_group_check=True)
                vk = pvk.tile([128, 512], fp32, name="vk")
                for b in range(B):
                    nc.tensor.matmul(vk[:, 128 * b:128 * (b + 1)],
                                     xslab[:, 128 * b:128 * (b + 1)], wv,
                                     start=(b == 0), stop=(b == 3),
                                     skip_group_check=True)

                # ---------------- phi ----------------
                e_k = work.tile([128, 512], bf16, name="e_k")
                nc.scalar.activation(e_k, u_k, Act.Exp)
                w_k = work.tile([128, 512], bf16, name="w_k")
                nc.vector.tensor_scalar_add(w_k, u_k, 1.0)
                e_q = work.tile([128, 512], bf16, name="e_q")
                nc.scalar.activation(e_q, u_q, Act.Exp)
                w_q = work.tile([128, 512], bf16, name="w_q")
                nc.scalar.activation(w_q, u_q, Act.Copy, bias=1.0)
                phi_k = work.tile([128, 512], bf16, name="phi_k")
                zch = work.tile([128, 4], fp32, name="zch")
                for b in range(B):
                    bs = slice(128 * b, 128 * (b + 1))
                    nc.vector.scalar_tensor_tensor(
                        phi_k[:, bs], e_k[:, bs], 1.0, w_k[:, bs],
                        Alu.min, Alu.max,
                        accum_out=None if last else zch[:, b:b + 1])
                phi_q = work.tile([128, 512], bf16, name="phi_q")
                nc.vector.scalar_tensor_tensor(
                    phi_q, e_q, 1.0, w_q, Alu.min, Alu.max)

                # vaug <- v (aug ones cols untouched)
                vsrc = vk.rearrange("p (g c) -> p g c", c=64)
                vdst = vaug.rearrange("p (g c) -> p g c", c=65)[:, :, 0:64]
                nc.scalar.copy(vdst, vsrc)

                # ---------------- knat via PE transpose ----------------
                if not last:
                    knp = pknp.tile([128, 512], bf16, name="knp")
                    for b in range(B):
                        bs = slice(128 * b, 128 * (b + 1))
                        nc.tensor.transpose(knp[:, bs], phi_k[:, bs], ident)

                # ------------- A^T = K Q^T per (b, h) --------------------
                # h0 shares the vk bank (all writers base partition 0);
                # h1 gets its own bank (sole writer, base 64)
                ah = [vk, pa1.tile([128, 512], fp32, name="ah1")]
                for h in range(2):
                    es = slice(64 * h, 64 * (h + 1))
                    for b in range(B):
                        bs = slice(128 * b, 128 * (b + 1))
                        nc.tensor.matmul(ah[h][:, bs], phi_k[es, bs],
                                         phi_q[es, bs], start=(b == 0),
                                         stop=(b == 3), skip_group_check=True)

                if not last:
                    knat = work.tile([128, 512], bf16, name="knat")
                    nc.vector.tensor_copy(knat, knp)

                am2 = [work.tile([128, 512], bf16, name=f"am2_{h}")
                       for h in range(2)]
                for h in range(2):
                    nc.vector.tensor_tensor(am2[h], ah[h], mask2, Alu.mult)

                # ---------------- num banks ----------------
                num = [pn0.tile([128, 260], fp32, name="num0"),
                       pn1.tile([128, 260], fp32, name="num1")]
                if i > 0:
                    # state-read: num_b[t, :] += sum_e q_b[e,t] [S|z]_b[e, :]
                    sp = s01[(i + 1) % 2]
                    for b in range(B):
                        nc.tensor.matmul(
                            num[b // 2][:, 130 * (b % 2):130 * (b % 2) + 130],
                            phi_q[:, 128 * b:128 * (b + 1)],
                            sp[:, 130 * b:130 * (b + 1)],
                            start=(b % 2 == 0), stop=False,
                            skip_group_check=True)
                # in-chunk: num += A_m^T [v | 1]
                for b in range(B):
                    for h in range(2):
                        co = 130 * (b % 2) + 65 * h
                        nc.tensor.matmul(
                            num[b // 2][:, co:co + 65],
                            am2[h][:, 128 * b:128 * (b + 1)],
                            vaug[:, 130 * b + 65 * h:130 * b + 65 * h + 65],
                            start=(i == 0 and h == 0 and b % 2 == 0),
                            stop=(h == 1 and b % 2 == 1),
                            skip_group_check=True)

                # ---------------- state update (one matmul per b) --------
                if not last:
                    vv = vaug.rearrange("p (g c) -> p g c", c=65)
                    for b in range(B):
                        nc.tensor.matmul(
                            st[:, 128 * b:128 * (b + 1)],
                            knat[:, 128 * b:128 * (b + 1)],
                            vv[:, 2 * b:2 * b + 2, 0:64],
                            start=False, stop=False, skip_group_check=True)
                    # evacuate: PSUM -> SBUF fp32 tmp (Scalar), then
                    # block-mask the cross-head garbage while casting to the
                    # s01 layout (GpSimd), and insert z columns (GpSimd)
                    nc.scalar.copy(stmp, st)
                    sc = s01[i % 2]
                    sdst = sc.rearrange("p (g c) -> p g c", c=65)[:, :, 0:64]
                    ssrc = stmp.rearrange("p (g c) -> p g c", c=64)
                    bm = bmask.rearrange("p (g c) -> p g c", c=64)
                    nc.gpsimd.tensor_tensor(sdst, ssrc, bm, Alu.mult)
                    zn = zsb[i % 2]
                    nc.vector.tensor_tensor(zn, zsb[(i + 1) % 2], zch, Alu.add)
                    sc3 = sc.rearrange("p (g c) -> p g c", c=130)
                    nc.gpsimd.tensor_copy(sc3[0:64, :, 64], zn[0:64, :])
                    nc.gpsimd.tensor_copy(sc3[64:128, :, 129], zn[64:128, :])

                # ---------------- export num/den ----------------
                nout0 = nouts.tile([128, 260], bf16, name="no0")
                nc.scalar.copy(nout0, num[0])
                nc.sync.dma_start(out_h[0, i], nout0)
                nout1 = nouts.tile([128, 260], bf16, name="no1")
                nc.vector.tensor_copy(nout1, num[1])
                nc.sync.dma_start(out_h[1, i], nout1)

    nc.finalize()
    return nc


def _host_prep(x, Wq, Wk, Wv, Wp):
    """Shard inputs per core; returns in_maps list."""
    x = np.asarray(x, dtype=np.float32)
    Wq = np.asarray(Wq, dtype=np.float32)
    Wk = np.asarray(Wk, dtype=np.float32)
    Wv = np.asarray(Wv, dtype=np.float32)
    Wp = np.asarray(Wp, dtype=np.float32)
    bf = ml_dtypes.bfloat16

    mask2 = np.tile(np.triu(np.ones((C, C), np.float32)), (1, 4)).astype(bf)
    # block mask: group g = (b, h): rows 64h..64h+64 are 1, others 0
    bmask = np.zeros((128, 512), np.float32)
    for g in range(8):
        h = g % 2
        bmask[64 * h:64 * (h + 1), 64 * g:64 * (g + 1)] = 1.0
    ident = np.eye(128, dtype=np.float32).astype(bf)
    ones = np.ones((1, 512), np.float32).astype(bf)
    zer = np.zeros((1, 512), np.float32).astype(bf)

    in_maps = []
    for c in range(NCORE):
        h0 = HPC * c
        xs = x[:, :, 64 * h0:64 * (h0 + HPC)]          # [B, S, 128]
        # xq[slab, feat, (lc, b, t)]
        xqa = xs.reshape(B, NSLAB, CPS, C, 128).transpose(1, 4, 2, 0, 3)
        xqa = np.ascontiguousarray(xqa).reshape(NSLAB, 128, CPS * 512)
        wq_bd = np.zeros((128, 128), np.float32)
        wk_bd = np.zeros((128, 128), np.float32)
        wv_bd = np.zeros((128, 128), np.float32)
        for j in range(HPC):
            h = h0 + j
            sl = slice(64 * j, 64 * (j + 1))
            wq_bd[sl, sl] = Wq[h]
            wk_bd[sl, sl] = Wk[h]
            wv_bd[sl, sl] = Wv[h] @ Wp[64 * h:64 * (h + 1), :]
        in_maps.append({
            "xq": xqa.astype(bf),
            "wq": wq_bd.astype(bf),
            "wk": wk_bd.astype(bf),
            "wv": wv_bd.astype(bf),
            "mask2": mask2,
            "bmask": bmask.astype(bf),
            "ident": ident,
            "ones": ones,
            "zer": zer,
        })
    return in_maps


def get_program():
    if "nc" not in _CACHE:
        _CACHE["nc"] = _build_program()
    return _CACHE["nc"]


def run_spmd(in_maps, **kwargs):
    from concourse.bass_utils import run_bass_kernel_spmd
    nc = get_program()
    return run_bass_kernel_spmd(nc, in_maps, list(range(NCORE)), **kwargs)


def kernel(x, Wq, Wk, Wv, Wp):
    in_maps = _host_prep(x, Wq, Wk, Wv, Wp)
    res = run_spmd(in_maps)
    out = np.zeros((B, S, O), np.float32)
    for c in range(NCORE):
        raw = np.asarray(res.results[c]["out"], dtype=np.float32)
        # raw[pr, i, t, 130j + (num_h0 64 | den_h0 | num_h1 64 | den_h1)]
        for pr in range(2):
            for j in range(2):
                b = 2 * pr + j
                nb = raw[pr, :, :, 130 * j:130 * (j + 1)].reshape(S, 130)
                out[b] += (nb[:, 0:64] / nb[:, 64:65]
                           + nb[:, 65:129] / nb[:, 129:130])
    return out


# revision 12
# speedup vs baseline: 1.6884x; 1.0287x over previous
"""Trainium2 Bass kernel for MultiLinearAttention (causal linear attention).

Reference computation (per head h, feature map phi(u) = elu(u)+1):
    q = phi(x_h @ Wq_h), k = phi(x_h @ Wk_h), v = x_h @ Wv_h
    y_t = (q_t . sum_{s<=t} k_s v_s^T) / (q_t . sum_{s<=t} k_s + eps)
    out = concat_h(y_h) @ Wp

Sharding: 16 heads / 8 cores = 2 heads per core, all 4 batches per core.
Wp is folded per-head into the v projection (W'_h = Wv_h @ Wp_h), so each
core produces per-head numerators num_h and denominators den_h; the host
computes sum_h num_h/den_h and sums partials over cores (the unshard step).

Device algorithm per chunk of C=128 (all 4 batches in one pass):
    u = W^T x  (PE);  e = exp(u), w = u+1  (Scalar/Vector)
    phi = max(min(e,1), w) == elu(proj)+1  (DVE, SBUF bf16 2x mode)
    A^T = K Q^T per (b,h) (PE, row-tiled by head), masked to s<=t (DVE)
    num = A_m^T [v|1] + Q_b^T [S|z]_b      (aug col gives den)
    S_b += K_b^T V_b: one matmul per batch over both heads; the cross-head
      blocks are garbage, zeroed by a block-mask multiply (GpSimd) during
      the PSUM->SBUF state evacuation. z is tracked via accum_out on the
      phi(k) ops and inserted into the evacuated state (GpSimd).
num/den are DMA'd out in bf16; the final division happens on the host.

HW constraint honored throughout: every matmul writing a given PSUM bank
reads its operands from the same base partition (mixing base 0 and base 64
writers in one bank faults on hardware; CoreSim does not model it).
"""

import os
import sys

import numpy as np

for _p in ("/root/.axon_site/_ro/trn_rl_repo", "/opt/trn_rl_repo", "/opt/pypackages"):
    if os.path.isdir(_p) and _p not in sys.path:
        sys.path.append(_p)

import ml_dtypes

B, S, D = 4, 4096, 1024
H, HD, O = 16, 64, 64
C = 128                  # chunk length
NCORE = 8
HPC = H // NCORE         # heads per core
NCHUNK = S // C
NSLAB = 8                # x is DMA'd in NSLAB slabs of NCHUNK//NSLAB chunks
CPS = NCHUNK // NSLAB    # chunks per slab

_CACHE = {}


def _build_program(nchunk=NCHUNK):
    import concourse.mybir as mybir
    from concourse import bacc
    from concourse.tile import TileContext

    fp32 = mybir.dt.float32
    bf16 = mybir.dt.bfloat16
    Alu = mybir.AluOpType
    Act = mybir.ActivationFunctionType

    nc = bacc.Bacc()
    xq_h = nc.declare_dram_parameter("xq", [NSLAB, 128, CPS * 512], bf16,
                                     isOutput=False)
    wq_h = nc.declare_dram_parameter("wq", [128, 128], bf16, isOutput=False)
    wk_h = nc.declare_dram_parameter("wk", [128, 128], bf16, isOutput=False)
    wv_h = nc.declare_dram_parameter("wv", [128, 128], bf16, isOutput=False)
    mask_h = nc.declare_dram_parameter("mask2", [128, 512], bf16, isOutput=False)
    bmask_h = nc.declare_dram_parameter("bmask", [128, 512], bf16, isOutput=False)
    ident_h = nc.declare_dram_parameter("ident", [128, 128], bf16, isOutput=False)
    ones_h = nc.declare_dram_parameter("ones", [1, 512], bf16, isOutput=False)
    zer_h = nc.declare_dram_parameter("zer", [1, 512], bf16, isOutput=False)
    # out[i] = [t, 520]: cols 130*b + [h0 num 64 | h0 den | h1 num 64
    # | h1 den] for batch b
    out_h = nc.declare_dram_parameter("out", [nchunk, 128, 520], bf16,
                                      isOutput=True)

    with TileContext(nc) as tc:
        with (
            tc.tile_pool(name="consts", bufs=1) as consts,
            tc.tile_pool(name="work", bufs=2) as work,
            tc.tile_pool(name="nouts", bufs=2) as nouts,
            tc.tile_pool(name="pk", bufs=1, space="PSUM") as pk,
            tc.tile_pool(name="pq", bufs=1, space="PSUM") as pq,
            tc.tile_pool(name="pvk", bufs=1, space="PSUM") as pvk,
            tc.tile_pool(name="pa1", bufs=1, space="PSUM") as pa1,
            tc.tile_pool(name="pknp", bufs=1, space="PSUM") as pknp,
            tc.tile_pool(name="pn0", bufs=1, space="PSUM") as pn0,
            tc.tile_pool(name="pn1", bufs=1, space="PSUM") as pn1,
            tc.tile_pool(name="pst", bufs=1, space="PSUM") as pst,
        ):
            # ---- constants into SBUF ----
            wq = consts.tile([128, 128], bf16)
            wk = consts.tile([128, 128], bf16)
            wv = consts.tile([128, 128], bf16)
            mask2 = consts.tile([128, 512], bf16)
            bmask = consts.tile([128, 512], bf16)
            ident = consts.tile([128, 128], bf16)
            ones = consts.tile([1, 512], bf16)
            zer = consts.tile([1, 512], bf16)
            nc.sync.dma_start(wq, wq_h[:, :])
            nc.sync.dma_start(wk, wk_h[:, :])
            nc.sync.dma_start(wv, wv_h[:, :])
            nc.sync.dma_start(mask2, mask_h[:, :])
            nc.sync.dma_start(bmask, bmask_h[:, :])
            nc.sync.dma_start(ident, ident_h[:, :])
            nc.sync.dma_start(ones, ones_h[:, :])
            nc.sync.dma_start(zer, zer_h[:, :])

            xsl = []
            for sb in range(NSLAB):
                t = consts.tile([128, CPS * 512], bf16, name=f"xq{sb}")
                nc.sync.dma_start(t, xq_h[sb])
                xsl.append(t)

            # persistent SBUF
            vaug = consts.tile([128, 520], bf16, name="vaug")
            nc.gpsimd.memset(vaug, 1.0)   # aug cols stay 1.0 forever
            # s01[j]: evacuated state, per b cols 130b + [S_h0|z_h0|S_h1|z_h1]
            # (block-diagonal per head; off-blocks stay zero)
            s01 = [consts.tile([128, 520], bf16, name=f"s01{j}")
                   for j in range(2)]
            for t in s01:
                nc.gpsimd.memset(t, 0.0)
            stmp = consts.tile([128, 512], fp32, name="stmp")
            one_t = consts.tile([128, 512], bf16, name="one_t")
            nc.gpsimd.memset(one_t, 1.0)
            zsb = [consts.tile([128, 4], fp32, name=f"z{j}") for j in range(2)]
            nc.gpsimd.memset(zsb[0], 0.0)
            nc.gpsimd.memset(zsb[1], 0.0)

            # persistent state PSUM bank [e, (b: h0 64 | h1 64)]; cross-head
            # blocks hold garbage (masked out during evacuation)
            st = pst.tile([128, 512], fp32, name="st")
            nc.tensor.matmul(st, ones[:, 0:128], zer[:, 0:512],
                             start=True, stop=False, skip_group_check=True)

            for i in range(nchunk):
                last = i == nchunk - 1
                xslab = xsl[i // CPS][:, (i % CPS) * 512:(i % CPS + 1) * 512]

                # ---------------- PE: projections ----------------
                u_k = pk.tile([128, 512], fp32, name="u_k")
                nc.tensor.matmul(u_k, wk, xslab, start=True, stop=True,
                                 skip_group_check=True)
                u_q = pq.tile([128, 512], fp32, name="u_q")
                nc.tensor.matmul(u_q, wq, xslab, start=True, stop=True,
                                 skip_group_check=True)
                vk = pvk.tile([128, 512], fp32, name="vk")
                for b in range(B):
                    nc.tensor.matmul(vk[:, 128 * b:128 * (b + 1)],
                                     xslab[:, 128 * b:128 * (b + 1)], wv,
                                     start=(b == 0), stop=(b == 3),
                                     skip_group_check=True)

                # ---------------- phi ----------------
                e_k = work.tile([128, 512], bf16, name="e_k")
                nc.scalar.activation(e_k, u_k, Act.Exp)
                w_k = work.tile([128, 512], bf16, name="w_k")
                nc.vector.tensor_scalar_add(w_k, u_k, 1.0)
                e_q = work.tile([128, 512], bf16, name="e_q")
                nc.scalar.activation(e_q, u_q, Act.Exp)
                w_q = work.tile([128, 512], bf16, name="w_q")
                nc.scalar.activation(w_q, u_q, Act.Copy, bias=1.0)
                phi_k = work.tile([128, 512], bf16, name="phi_k")
                zch = work.tile([128, 4], fp32, name="zch")
                for b in range(B):
                    bs = slice(128 * b, 128 * (b + 1))
                    nc.vector.scalar_tensor_tensor(
                        phi_k[:, bs], e_k[:, bs], 1.0, w_k[:, bs],
                        Alu.min, Alu.max,
                        accum_out=None if last else zch[:, b:b + 1])
                phi_q = work.tile([128, 512], bf16, name="phi_q")
                nc.vector.scalar_tensor_tensor(
                    phi_q, e_q, 1.0, w_q, Alu.min, Alu.max)

                # vaug <- v (aug ones cols untouched)
                vsrc = vk.rearrange("p (g c) -> p g c", c=64)
                vdst = vaug.rearrange("p (g c) -> p g c", c=65)[:, :, 0:64]
                nc.scalar.copy(vdst, vsrc)

                # ---------------- knat via PE transpose ----------------
                if not last:
                    knp = pknp.tile([128, 512], bf16, name="knp")
                    for b in range(B):
                        bs = slice(128 * b, 128 * (b + 1))
                        nc.tensor.transpose(knp[:, bs], phi_k[:, bs], ident)

                # ------------- A^T = K Q^T per (b, h) --------------------
                # h0 shares the vk bank (all writers base partition 0);
                # h1 gets its own bank (sole writer, base 64)
                ah = [vk, pa1.tile([128, 512], fp32, name="ah1")]
                for h in range(2):
                    es = slice(64 * h, 64 * (h + 1))
                    for b in range(B):
                        bs = slice(128 * b, 128 * (b + 1))
                        nc.tensor.matmul(ah[h][:, bs], phi_k[es, bs],
                                         phi_q[es, bs], start=(b == 0),
                                         stop=(b == 3), skip_group_check=True)

                if not last:
                    knat = work.tile([128, 512], bf16, name="knat")
                    nc.vector.tensor_copy(knat, knp)

                am2 = [work.tile([128, 512], bf16, name=f"am2_{h}")
                       for h in range(2)]
                for h in range(2):
                    nc.vector.tensor_tensor(am2[h], ah[h], mask2, Alu.mult)

                # ---------------- state update (one matmul per b) --------
                # (emitted before the num matmuls: the evacuation chain
                # S->GP->GP must finish before next chunk's state-read)
                if not last:
                    vv = vaug.rearrange("p (g c) -> p g c", c=65)
                    for b in range(B):
                        nc.tensor.matmul(
                            st[:, 128 * b:128 * (b + 1)],
                            knat[:, 128 * b:128 * (b + 1)],
                            vv[:, 2 * b:2 * b + 2, 0:64],
                            start=False, stop=False, skip_group_check=True)
                    # evacuate: PSUM -> SBUF fp32 tmp (Scalar), then
                    # block-mask the cross-head garbage while casting to the
                    # s01 layout (GpSimd), and insert z columns (GpSimd)
                    nc.scalar.copy(stmp, st)
                    sc = s01[i % 2]
                    sdst = sc.rearrange("p (g c) -> p g c", c=65)[:, :, 0:64]
                    ssrc = stmp.rearrange("p (g c) -> p g c", c=64)
                    bm = bmask.rearrange("p (g c) -> p g c", c=64)
                    nc.gpsimd.tensor_tensor(sdst, ssrc, bm, Alu.mult)
                    zn = zsb[i % 2]
                    nc.gpsimd.tensor_tensor(zn, zsb[(i + 1) % 2], zch, Alu.add)
                    sc3 = sc.rearrange("p (g c) -> p g c", c=130)
                    nc.gpsimd.tensor_copy(sc3[0:64, :, 64], zn[0:64, :])
                    nc.gpsimd.tensor_copy(sc3[64:128, :, 129], zn[64:128, :])

                # ---------------- num banks ----------------
                num = [pn0.tile([128, 260], fp32, name="num0"),
                       pn1.tile([128, 260], fp32, name="num1")]
                if i > 0:
                    # state-read: num_b[t, :] += sum_e q_b[e,t] [S|z]_b[e, :]
                    sp = s01[(i + 1) % 2]
                    for b in range(B):
                        nc.tensor.matmul(
                            num[b // 2][:, 130 * (b % 2):130 * (b % 2) + 130],
                            phi_q[:, 128 * b:128 * (b + 1)],
                            sp[:, 130 * b:130 * (b + 1)],
                            start=(b % 2 == 0), stop=False,
                            skip_group_check=True)
                # in-chunk: num += A_m^T [v | 1]
                for b in range(B):
                    for h in range(2):
                        co = 130 * (b % 2) + 65 * h
                        nc.tensor.matmul(
                            num[b // 2][:, co:co + 65],
                            am2[h][:, 128 * b:128 * (b + 1)],
                            vaug[:, 130 * b + 65 * h:130 * b + 65 * h + 65],
                            start=(i == 0 and h == 0 and b % 2 == 0),
                            stop=(h == 1 and b % 2 == 1),
                            skip_group_check=True)

                # ---------------- export num/den ----------------
                nout = nouts.tile([128, 520], bf16, name="no")
                nc.scalar.copy(nout[:, 0:260], num[0])
                nc.vector.tensor_copy(nout[:, 260:520], num[1])
                nc.sync.dma_start(out_h[i], nout)

    nc.finalize()
    return nc


def _host_prep(x, Wq, Wk, Wv, Wp):
    """Shard inputs per core; returns in_maps list."""
    x = np.asarray(x, dtype=np.float32)
    Wq = np.asarray(Wq, dtype=np.float32)
    Wk = np.asarray(Wk, dtype=np.float32)
    Wv = np.asarray(Wv, dtype=np.float32)
    Wp = np.asarray(Wp, dtype=np.float32)
    bf = ml_dtypes.bfloat16

    mask2 = np.tile(np.triu(np.ones((C, C), np.float32)), (1, 4)).astype(bf)
    # block mask: group g = (b, h): rows 64h..64h+64 are 1, others 0
    bmask = np.zeros((128, 512), np.float32)
    for g in range(8):
        h = g % 2
        bmask[64 * h:64 * (h + 1), 64 * g:64 * (g + 1)] = 1.0
    ident = np.eye(128, dtype=np.float32).astype(bf)
    ones = np.ones((1, 512), np.float32).astype(bf)
    zer = np.zeros((1, 512), np.float32).astype(bf)

    in_maps = []
    for c in range(NCORE):
        h0 = HPC * c
        xs = x[:, :, 64 * h0:64 * (h0 + HPC)]          # [B, S, 128]
        # xq[slab, feat, (lc, b, t)]
        xqa = xs.reshape(B, NSLAB, CPS, C, 128).transpose(1, 4, 2, 0, 3)
        xqa = np.ascontiguousarray(xqa).reshape(NSLAB, 128, CPS * 512)
        wq_bd = np.zeros((128, 128), np.float32)
        wk_bd = np.zeros((128, 128), np.float32)
        wv_bd = np.zeros((128, 128), np.float32)
        for j in range(HPC):
            h = h0 + j
            sl = slice(64 * j, 64 * (j + 1))
            wq_bd[sl, sl] = Wq[h]
            wk_bd[sl, sl] = Wk[h]
            wv_bd[sl, sl] = Wv[h] @ Wp[64 * h:64 * (h + 1), :]
        in_maps.append({
            "xq": xqa.astype(bf),
            "wq": wq_bd.astype(bf),
            "wk": wk_bd.astype(bf),
            "wv": wv_bd.astype(bf),
            "mask2": mask2,
            "bmask": bmask.astype(bf),
            "ident": ident,
            "ones": ones,
            "zer": zer,
        })
    return in_maps


def get_program():
    if "nc" not in _CACHE:
        _CACHE["nc"] = _build_program()
    return _CACHE["nc"]


def run_spmd(in_maps, **kwargs):
    from concourse.bass_utils import run_bass_kernel_spmd
    nc = get_program()
    return run_bass_kernel_spmd(nc, in_maps, list(range(NCORE)), **kwargs)


def kernel(x, Wq, Wk, Wv, Wp):
    in_maps = _host_prep(x, Wq, Wk, Wv, Wp)
    res = run_spmd(in_maps)
    out = np.zeros((B, S, O), np.float32)
    for c in range(NCORE):
        raw = np.asarray(res.results[c]["out"], dtype=np.float32)
        # raw[i, t, 130b + (num_h0 64 | den_h0 | num_h1 64 | den_h1)]
        for b in range(B):
            nb = raw[:, :, 130 * b:130 * (b + 1)].reshape(S, 130)
            out[b] += (nb[:, 0:64] / nb[:, 64:65]
                       + nb[:, 65:129] / nb[:, 129:130])
    return out


# revision 16
# speedup vs baseline: 1.7079x; 1.0116x over previous
"""Trainium2 Bass kernel for MultiLinearAttention (causal linear attention).

Reference computation (per head h, feature map phi(u) = elu(u)+1):
    q = phi(x_h @ Wq_h), k = phi(x_h @ Wk_h), v = x_h @ Wv_h
    y_t = (q_t . sum_{s<=t} k_s v_s^T) / (q_t . sum_{s<=t} k_s + eps)
    out = concat_h(y_h) @ Wp

Sharding: 16 heads / 8 cores = 2 heads per core, all 4 batches per core.
Wp is folded per-head into the v projection (W'_h = Wv_h @ Wp_h), so each
core produces per-head numerators num_h and denominators den_h; the host
computes sum_h num_h/den_h and sums partials over cores (the unshard step).

Device algorithm per chunk of C=128 (all 4 batches in one pass):
    u = W^T x  (PE);  e = exp(u), w = u+1  (Scalar/Vector)
    phi = max(min(e,1), w) == elu(proj)+1  (DVE, SBUF bf16 2x mode)
    A^T = K Q^T per (b,h) (PE, row-tiled by head), masked to s<=t (DVE)
    num = A_m^T [v|1] + Q_b^T [S|z]_b      (aug col gives den)
    S_b += K_b^T V_b: one matmul per batch over both heads; the cross-head
      blocks are garbage, zeroed by a block-mask multiply (GpSimd) during
      the PSUM->SBUF state evacuation. z is tracked via accum_out on the
      phi(k) ops and inserted into the evacuated state (GpSimd).
num/den are DMA'd out in bf16; the final division happens on the host.

HW constraint honored throughout: every matmul writing a given PSUM bank
reads its operands from the same base partition (mixing base 0 and base 64
writers in one bank faults on hardware; CoreSim does not model it).
"""

import os
import sys

import numpy as np

for _p in ("/root/.axon_site/_ro/trn_rl_repo", "/opt/trn_rl_repo", "/opt/pypackages"):
    if os.path.isdir(_p) and _p not in sys.path:
        sys.path.append(_p)

import ml_dtypes

B, S, D = 4, 4096, 1024
H, HD, O = 16, 64, 64
C = 128                  # chunk length
NCORE = 8
HPC = H // NCORE         # heads per core
NCHUNK = S // C
NSLAB = 8                # x is DMA'd in NSLAB slabs of NCHUNK//NSLAB chunks
CPS = NCHUNK // NSLAB    # chunks per slab

_CACHE = {}


def _build_program(nchunk=NCHUNK):
    import concourse.mybir as mybir
    from concourse import bacc
    from concourse.tile import TileContext

    fp32 = mybir.dt.float32
    bf16 = mybir.dt.bfloat16
    Alu = mybir.AluOpType
    Act = mybir.ActivationFunctionType

    nc = bacc.Bacc()
    xq_h = nc.declare_dram_parameter("xq", [NSLAB, 128, CPS * 512], bf16,
                                     isOutput=False)
    wq_h = nc.declare_dram_parameter("wq", [128, 128], bf16, isOutput=False)
    wk_h = nc.declare_dram_parameter("wk", [128, 128], bf16, isOutput=False)
    wv_h = nc.declare_dram_parameter("wv", [128, 128], bf16, isOutput=False)
    mask_h = nc.declare_dram_parameter("mask2", [128, 512], bf16, isOutput=False)
    bmask_h = nc.declare_dram_parameter("bmask", [128, 512], bf16, isOutput=False)
    ident_h = nc.declare_dram_parameter("ident", [128, 128], bf16, isOutput=False)
    ones_h = nc.declare_dram_parameter("ones", [1, 512], bf16, isOutput=False)
    zer_h = nc.declare_dram_parameter("zer", [1, 512], bf16, isOutput=False)
    # out[i] = [t, 520]: cols 130*b + [h0 num 64 | h0 den | h1 num 64
    # | h1 den] for batch b
    out_h = nc.declare_dram_parameter("out", [nchunk, 128, 520], bf16,
                                      isOutput=True)

    with TileContext(nc) as tc:
        with (
            tc.tile_pool(name="consts", bufs=1) as consts,
            tc.tile_pool(name="work", bufs=2) as work,
            tc.tile_pool(name="nouts", bufs=2) as nouts,
            tc.tile_pool(name="pqk", bufs=1, space="PSUM") as pqk,
            tc.tile_pool(name="pvk", bufs=1, space="PSUM") as pvk,
            tc.tile_pool(name="pa1", bufs=1, space="PSUM") as pa1,
            tc.tile_pool(name="pknp", bufs=1, space="PSUM") as pknp,
            tc.tile_pool(name="pn0", bufs=1, space="PSUM") as pn0,
            tc.tile_pool(name="pn1", bufs=1, space="PSUM") as pn1,
            tc.tile_pool(name="pst", bufs=1, space="PSUM") as pst,
        ):
            # ---- constants into SBUF ----
            wq = consts.tile([128, 128], bf16)
            wk = consts.tile([128, 128], bf16)
            wv = consts.tile([128, 128], bf16)
            mask2 = consts.tile([128, 512], bf16)
            bmask = consts.tile([128, 512], bf16)
            ident = consts.tile([128, 128], bf16)
            ones = consts.tile([1, 512], bf16)
            zer = consts.tile([1, 512], bf16)
            nc.sync.dma_start(wq, wq_h[:, :])
            nc.sync.dma_start(wk, wk_h[:, :])
            nc.sync.dma_start(wv, wv_h[:, :])
            nc.sync.dma_start(mask2, mask_h[:, :])
            nc.sync.dma_start(bmask, bmask_h[:, :])
            nc.sync.dma_start(ident, ident_h[:, :])
            nc.sync.dma_start(ones, ones_h[:, :])
            nc.sync.dma_start(zer, zer_h[:, :])

            xsl = []
            for sb in range(NSLAB):
                t = consts.tile([128, CPS * 512], bf16, name=f"xq{sb}")
                nc.sync.dma_start(t, xq_h[sb])
                xsl.append(t)

            # persistent SBUF
            vaug = consts.tile([128, 520], bf16, name="vaug")
            nc.gpsimd.memset(vaug, 1.0)   # aug cols stay 1.0 forever
            # s01[j]: evacuated state, per b cols 130b + [S_h0|z_h0|S_h1|z_h1]
            # (block-diagonal per head; off-blocks stay zero)
            s01 = [consts.tile([128, 520], bf16, name=f"s01{j}")
                   for j in range(2)]
            for t in s01:
                nc.gpsimd.memset(t, 0.0)
            stmp = consts.tile([128, 512], bf16, name="stmp")
            zsb = [consts.tile([128, 4], fp32, name=f"z{j}") for j in range(2)]
            nc.gpsimd.memset(zsb[0], 0.0)
            nc.gpsimd.memset(zsb[1], 0.0)

            # persistent state PSUM bank [e, (b: h0 64 | h1 64)]; cross-head
            # blocks hold garbage (masked out during evacuation)
            st = pst.tile([128, 512], fp32, name="st")
            nc.tensor.matmul(st, ones[:, 0:128], zer[:, 0:512],
                             start=True, stop=False, skip_group_check=True)

            for i in range(nchunk):
                last = i == nchunk - 1
                xslab = xsl[i // CPS][:, (i % CPS) * 512:(i % CPS + 1) * 512]

                # ---------------- PE: projections ----------------
                # u2 spans two PSUM banks: [k-bank | q-bank]; each matmul
                # writes within one bank
                u2 = pqk.tile([128, 1024], fp32, name="u2")
                u_k = u2[:, 0:512]
                u_q = u2[:, 512:1024]
                nc.tensor.matmul(u_k, wk, xslab, start=True, stop=True,
                                 skip_group_check=True)
                nc.tensor.matmul(u_q, wq, xslab, start=True, stop=True,
                                 skip_group_check=True)
                vk = pvk.tile([128, 512], fp32, name="vk")
                for b in range(B):
                    nc.tensor.matmul(vk[:, 128 * b:128 * (b + 1)],
                                     xslab[:, 128 * b:128 * (b + 1)], wv,
                                     start=(b == 0), stop=(b == 3),
                                     skip_group_check=True)

                # ---------------- phi ----------------
                e2 = work.tile([128, 1024], bf16, name="e2")
                nc.scalar.activation(e2, u2, Act.Exp)
                w2 = work.tile([128, 1024], bf16, name="w2")
                nc.scalar.activation(w2, u2, Act.Copy, bias=1.0)
                phi_k = work.tile([128, 512], bf16, name="phi_k")
                zch = work.tile([128, 4], fp32, name="zch")
                for b in range(B):
                    bs = slice(128 * b, 128 * (b + 1))
                    nc.vector.scalar_tensor_tensor(
                        phi_k[:, bs], e2[:, bs], 1.0, w2[:, bs],
                        Alu.min, Alu.max,
                        accum_out=None if last else zch[:, b:b + 1])
                phi_q = work.tile([128, 512], bf16, name="phi_q")
                nc.vector.scalar_tensor_tensor(
                    phi_q, e2[:, 512:1024], 1.0, w2[:, 512:1024],
                    Alu.min, Alu.max)

                # vaug <- v (aug ones cols untouched)
                vsrc = vk.rearrange("p (g c) -> p g c", c=64)
                vdst = vaug.rearrange("p (g c) -> p g c", c=65)[:, :, 0:64]
                nc.scalar.copy(vdst, vsrc)

                # ---------------- knat via PE transpose ----------------
                if not last:
                    knp = pknp.tile([128, 512], bf16, name="knp")
                    for b in range(B):
                        bs = slice(128 * b, 128 * (b + 1))
                        nc.tensor.transpose(knp[:, bs], phi_k[:, bs], ident)

                # ------------- A^T = K Q^T per (b, h) --------------------
                # h0 shares the vk bank (all writers base partition 0);
                # h1 gets its own bank (sole writer, base 64)
                ah = [vk, pa1.tile([128, 512], fp32, name="ah1")]
                for h in range(2):
                    es = slice(64 * h, 64 * (h + 1))
                    for b in range(B):
                        bs = slice(128 * b, 128 * (b + 1))
                        nc.tensor.matmul(ah[h][:, bs], phi_k[es, bs],
                                         phi_q[es, bs], start=(b == 0),
                                         stop=(b == 3), skip_group_check=True)

                if not last:
                    knat = work.tile([128, 512], bf16, name="knat")
                    nc.vector.tensor_copy(knat, knp)

                am2 = [work.tile([128, 512], bf16, name=f"am2_{h}")
                       for h in range(2)]
                for h in range(2):
                    nc.vector.tensor_tensor(am2[h], ah[h], mask2, Alu.mult)

                # ---------------- state update (one matmul per b) --------
                # (emitted before the num matmuls: the evacuation chain
                # S->GP->GP must finish before next chunk's state-read)
                if not last:
                    vv = vaug.rearrange("p (g c) -> p g c", c=65)
                    for b in range(B):
                        nc.tensor.matmul(
                            st[:, 128 * b:128 * (b + 1)],
                            knat[:, 128 * b:128 * (b + 1)],
                            vv[:, 2 * b:2 * b + 2, 0:64],
                            start=False, stop=False, skip_group_check=True)
                    # evacuate: PSUM -> SBUF fp32 tmp (Scalar), then
                    # block-mask the cross-head garbage while casting to the
                    # s01 layout (GpSimd), and insert z columns (GpSimd)
                    nc.scalar.copy(stmp, st)
                    sc = s01[i % 2]
                    sdst = sc.rearrange("p (g c) -> p g c", c=65)[:, :, 0:64]
                    ssrc = stmp.rearrange("p (g c) -> p g c", c=64)
                    bm = bmask.rearrange("p (g c) -> p g c", c=64)
                    nc.gpsimd.tensor_tensor(sdst, ssrc, bm, Alu.mult)
                    zn = zsb[i % 2]
                    nc.vector.tensor_tensor(zn, zsb[(i + 1) % 2], zch, Alu.add)
                    sc3 = sc.rearrange("p (g c) -> p g c", c=130)
                    nc.gpsimd.tensor_copy(sc3[0:64, :, 64], zn[0:64, :])
                    nc.gpsimd.tensor_copy(sc3[64:128, :, 129], zn[64:128, :])

                # ---------------- num banks ----------------
                num = [pn0.tile([128, 260], fp32, name="num0"),
                       pn1.tile([128, 260], fp32, name="num1")]
                if i > 0:
                    # state-read: num_b[t, :] += sum_e q_b[e,t] [S|z]_b[e, :]
                    sp = s01[(i + 1) % 2]
                    for b in range(B):
                        nc.tensor.matmul(
                            num[b // 2][:, 130 * (b % 2):130 * (b % 2) + 130],
                            phi_q[:, 128 * b:128 * (b + 1)],
                            sp[:, 130 * b:130 * (b + 1)],
                            start=(b % 2 == 0), stop=False,
                            skip_group_check=True)
                # in-chunk: num += A_m^T [v | 1]
                for b in range(B):
                    for h in range(2):
                        co = 130 * (b % 2) + 65 * h
                        nc.tensor.matmul(
                            num[b // 2][:, co:co + 65],
                            am2[h][:, 128 * b:128 * (b + 1)],
                            vaug[:, 130 * b + 65 * h:130 * b + 65 * h + 65],
                            start=(i == 0 and h == 0 and b % 2 == 0),
                            stop=(h == 1 and b % 2 == 1),
                            skip_group_check=True)

                # ---------------- export num/den ----------------
                nout = nouts.tile([128, 520], bf16, name="no")
                nc.scalar.copy(nout[:, 0:260], num[0])
                nc.vector.tensor_copy(nout[:, 260:520], num[1])
                nc.sync.dma_start(out_h[i], nout)

    nc.finalize()
    return nc


def _host_prep(x, Wq, Wk, Wv, Wp):
    """Shard inputs per core; returns in_maps list."""
    x = np.asarray(x, dtype=np.float32)
    Wq = np.asarray(Wq, dtype=np.float32)
    Wk = np.asarray(Wk, dtype=np.float32)
    Wv = np.asarray(Wv, dtype=np.float32)
    Wp = np.asarray(Wp, dtype=np.float32)
    bf = ml_dtypes.bfloat16

    mask2 = np.tile(np.triu(np.ones((C, C), np.float32)), (1, 4)).astype(bf)
    # block mask: group g = (b, h): rows 64h..64h+64 are 1, others 0
    bmask = np.zeros((128, 512), np.float32)
    for g in range(8):
        h = g % 2
        bmask[64 * h:64 * (h + 1), 64 * g:64 * (g + 1)] = 1.0
    ident = np.eye(128, dtype=np.float32).astype(bf)
    ones = np.ones((1, 512), np.float32).astype(bf)
    zer = np.zeros((1, 512), np.float32).astype(bf)

    in_maps = []
    for c in range(NCORE):
        h0 = HPC * c
        xs = x[:, :, 64 * h0:64 * (h0 + HPC)]          # [B, S, 128]
        # xq[slab, feat, (lc, b, t)]
        xqa = xs.reshape(B, NSLAB, CPS, C, 128).transpose(1, 4, 2, 0, 3)
        xqa = np.ascontiguousarray(xqa).reshape(NSLAB, 128, CPS * 512)
        wq_bd = np.zeros((128, 128), np.float32)
        wk_bd = np.zeros((128, 128), np.float32)
        wv_bd = np.zeros((128, 128), np.float32)
        for j in range(HPC):
            h = h0 + j
            sl = slice(64 * j, 64 * (j + 1))
            wq_bd[sl, sl] = Wq[h]
            wk_bd[sl, sl] = Wk[h]
            wv_bd[sl, sl] = Wv[h] @ Wp[64 * h:64 * (h + 1), :]
        in_maps.append({
            "xq": xqa.astype(bf),
            "wq": wq_bd.astype(bf),
            "wk": wk_bd.astype(bf),
            "wv": wv_bd.astype(bf),
            "mask2": mask2,
            "bmask": bmask.astype(bf),
            "ident": ident,
            "ones": ones,
            "zer": zer,
        })
    return in_maps


def get_program():
    if "nc" not in _CACHE:
        _CACHE["nc"] = _build_program()
    return _CACHE["nc"]


def run_spmd(in_maps, **kwargs):
    from concourse.bass_utils import run_bass_kernel_spmd
    nc = get_program()
    return run_bass_kernel_spmd(nc, in_maps, list(range(NCORE)), **kwargs)


def kernel(x, Wq, Wk, Wv, Wp):
    in_maps = _host_prep(x, Wq, Wk, Wv, Wp)
    res = run_spmd(in_maps)
    out = np.zeros((B, S, O), np.float32)
    for c in range(NCORE):
        raw = np.asarray(res.results[c]["out"], dtype=np.float32)
        # raw[i, t, 130b + (num_h0 64 | den_h0 | num_h1 64 | den_h1)]
        for b in range(B):
            nb = raw[:, :, 130 * b:130 * (b + 1)].reshape(S, 130)
            out[b] += (nb[:, 0:64] / nb[:, 64:65]
                       + nb[:, 65:129] / nb[:, 129:130])
    return out
